# revision 38
# baseline (speedup 1.0000x reference)
"""Multi-head causal attention (B=2, N=2048, D=1024, H=16) on 8 TRN2 NeuronCores.

Sharding: data-parallel over batch (2) x tensor-parallel over head groups (4),
so each core handles one batch element and 4 heads (256 of the 1024 hidden
channels). Wq/Wk/Wv are column-sharded, Wo row-sharded; each core emits a
partial output [2048, 1024] (bf16) that the host sums over the 4 head groups.

Per-core dataflow (matmuls bf16 with fp32 PSUM accumulation), fully
software-pipelined so the PE never idles (which would drop it out of the
2.4 GHz p-state):

  Q^T/K^T/V projections and the output projection are *interleaved into the
  attention loop* as filler units between S/PV iterations -- chunk qc's
  attention runs while chunk qc+1's Q/K and upcoming V seq-tiles project and
  chunk qc-1's output tiles drain.

  S^T[k,q] per k-tile: two K=64 matmuls at PE row groups h0/h64 (they execute
  concurrently on the PE array), trimmed at the causal diagonal (a diagonal
  k-tile with offset di only computes q-columns [128*di, 512)).

  exp on the Scalar engine covers both heads' trimmed regions in one strided
  ACTIVATE; causal masking multiplies only the [128,128] boundary subtile by
  a single shared triangular mask.

  PV (U += V_aug^T expS) lags S by 2 k-tiles; V is stored per seq-tile with
  ones/zeros scaffolding so the softmax denominator accumulates as an extra
  U row (even head: partition 64, odd head: partition 0).

  Normalize per (chunk, head-pair): the two denominator rows are copied to
  SBUF (bf16), partition-broadcast with two K=1 rank-1 matmuls into a PSUM
  tile, reciprocal'd in one DVE op, then ctx^T = U * (1/r).  No DRAM bounce.

  Y = ctx^T^T Wo emitted per seq-tile half as filler during later chunks,
  cast to bf16 and DMA'd out (host sums partials in fp32).
"""

import sys

sys.path.insert(0, "/opt/trn_rl_repo")

from collections import deque

import numpy as np
import ml_dtypes

import concourse.bass as bass
import concourse.bacc as bacc
import concourse.mybir as mybir
from concourse.tile import TileContext
from concourse.bass_utils import run_bass_kernel_spmd

BF16 = mybir.dt.bfloat16
F32 = mybir.dt.float32

B, N, D, H = 2, 2048, 1024, 16
HD = 64          # head dim
HPC = 4          # heads per core
DH = HPC * HD    # 256 hidden channels per core
NCORES = 8
KT = D // 128    # 8 contraction tiles over D
ST = N // 128    # 16 seq tiles
QC = N // 512    # 4 q-chunks of 512
LAG = 2          # PV lags S by this many k-tiles

# v_sb per-seq-tile column layout: for each head pair, an "even" block
# [V(64) | ones(1)] (matmul M=65 -> U partitions 0..64, denom at 64) and an
# "odd" block [ones(1) | zeros(63) | V(64)] (M=128 -> U partitions 64..127
# hold data, denom at partition 0, zeros keep partitions 1..63 inert).
V_BLK = {0: (0, 65), 1: (65, 193), 2: (193, 258), 3: (258, 386)}
V_COLS = 386


def _build_nc() -> bass.Bass:
    nc = bacc.Bacc()
    xT = nc.declare_dram_parameter("xT", [D, N], BF16, isOutput=False)
    wq = nc.declare_dram_parameter("wq", [D, DH], BF16, isOutput=False)
    wk = nc.declare_dram_parameter("wk", [D, DH], BF16, isOutput=False)
    wv = nc.declare_dram_parameter("wv", [D, DH], BF16, isOutput=False)
    wo = nc.declare_dram_parameter("wo", [DH, D], BF16, isOutput=False)
    y = nc.declare_dram_parameter("y", [N, D], BF16, isOutput=True)

    xT_r = xT.rearrange("(t p) n -> t p n", p=128)
    wq_r = wq.rearrange("(t p) m -> t p m", p=128)
    wk_r = wk.rearrange("(t p) m -> t p m", p=128)
    wv_r = wv.rearrange("(t p) m -> t p m", p=128)
    wo_r = wo.rearrange("(t p) m -> t p m", p=128)
    y_r = y.rearrange("(t p) m -> t p m", p=128)

    with TileContext(nc) as tc:
        with (
            tc.tile_pool(name="const", bufs=1) as cpool,
            tc.tile_pool(name="io", bufs=3) as io_pool,
            tc.tile_pool(name="exps", bufs=10) as exp_pool,
            tc.tile_pool(name="small", bufs=2) as small_pool,
            tc.tile_pool(name="ps_s", bufs=2, space="PSUM") as ps_s_pool,
            tc.tile_pool(name="ps_u", bufs=2, space="PSUM") as ps_u_pool,
            tc.tile_pool(name="ps_m", bufs=2, space="PSUM") as ps_m_pool,
        ):
            xT_sb = cpool.tile([128, KT, N], BF16)
            wq_sb = cpool.tile([128, KT, DH], BF16)
            wk_sb = cpool.tile([128, KT, DH], BF16)
            wv_sb = cpool.tile([128, KT, DH], BF16)
            wo_sb = cpool.tile([128, 2, D], BF16)
            qT_sb = cpool.tile([128, 2, N], BF16)
            kT_sb = cpool.tile([128, 2, N], BF16)
            v_sb = cpool.tile([128, ST, V_COLS], BF16)
            ctxT_sb = cpool.tile([128, 2, N], BF16)
            mask_sb = cpool.tile([128, 128], BF16)
            ones_sb = cpool.tile([128, 512], BF16)
            # chunk-3 exp tiles for k-tiles 0-7 (both head pairs), precomputed
            # as fillers during chunks 1-2 where the Scalar engine idles
            ex3_sb = cpool.tile([128, 16, 1024], BF16)

            # Shared [128,128] causal boundary mask: keep (1.0) where q >= k.
            # Emitted before the xT DMAs so the gpsimd queue produces it early.
            nc.vector.memset(mask_sb, 1.0)
            nc.gpsimd.affine_select(
                out=mask_sb,
                in_=mask_sb,
                compare_op=mybir.AluOpType.is_ge,
                fill=0.0,
                base=0,
                pattern=[[1, 128]],
                channel_multiplier=-1,
            )
            nc.vector.memset(ones_sb, 1.0)
            # ones / zeros scaffolding of the V blocks (all seq tiles at once)
            nc.vector.memset(v_sb[:, :, 66:129], 0.0)
            nc.vector.memset(v_sb[:, :, 259:322], 0.0)
            for col in (64, 65, 257, 258):
                nc.vector.memset(v_sb[:, :, col : col + 1], 1.0)

            # Weights stream on the sync DMA queue; xT streams (chunk-0
            # quarters first, then the rest) on the gpsimd queue.  The two
            # queues run concurrently.  Few, large DMAs: each call has a
            # ~0.6us descriptor floor.
            def w_dram_ap(w_r, t0, nt):
                base = w_r[t0]
                return bass.AP(
                    tensor=base.tensor,
                    offset=base.offset,
                    ap=[base.ap[0], [128 * DH, nt], [1, DH]],
                )

            def w_sb_ap(w_sb_t, t0, nt):
                return bass.AP(
                    tensor=w_sb_t.tensor,
                    offset=w_sb_t[:, t0, 0:1].offset,
                    ap=[w_sb_t.ap[0], [DH, nt], [1, DH]],
                )

            # spread the input stream over three DMA-capable queues (each
            # queue sustains only ~195 GB/s and has a ~0.6us per-call floor):
            # sync carries wq + odd xT tiles, scalar wk, gpsimd even xT
            # tiles + wv + wo
            def xt_dma(q_eng, t, qc):
                q_eng.dma_start(
                    out=xT_sb[:, t, 512 * qc : 512 * (qc + 1)],
                    in_=xT_r[t][:, 512 * qc : 512 * (qc + 1)],
                )

            def w_half_dram(w_r, mt):
                base = w_r[0]
                return bass.AP(
                    tensor=base.tensor,
                    offset=base.offset + 128 * mt,
                    ap=[base.ap[0], [128 * DH, KT], [1, 128]],
                )

            def w_half_sb(w_sb_t, mt):
                return bass.AP(
                    tensor=w_sb_t.tensor,
                    offset=w_sb_t[:, 0, 128 * mt : 128 * mt + 1].offset,
                    ap=[w_sb_t.ap[0], [DH, KT], [1, 128]],
                )

            # the head-pair-0 halves of Wq/Wk gate the first projections --
            # land them (and chunk-0 of xT) before everything else
            nc.sync.dma_start(out=w_half_sb(wq_sb, 0), in_=w_half_dram(wq_r, 0))
            nc.scalar.dma_start(
                out=w_half_sb(wk_sb, 0), in_=w_half_dram(wk_r, 0)
            )
            for t in range(1, KT, 2):
                xt_dma(nc.sync, t, 0)
            for t in range(0, KT, 2):
                xt_dma(nc.gpsimd, t, 0)
            nc.sync.dma_start(out=w_half_sb(wq_sb, 1), in_=w_half_dram(wq_r, 1))
            nc.scalar.dma_start(
                out=w_half_sb(wk_sb, 1), in_=w_half_dram(wk_r, 1)
            )
            nc.gpsimd.dma_start(
                out=w_sb_ap(wv_sb, 0, 8), in_=w_dram_ap(wv_r, 0, 8)
            )
            for t in range(2):
                nc.gpsimd.dma_start(out=wo_sb[:, t, :], in_=wo_r[t])

            def emit_xt_rest():
                # gate: the c1-c3 xT transfers start only after this memset
                # (emitted after the preamble) so they don't steal shared
                # AXI bandwidth from the chunk-0 critical-path transfers
                nc.vector.memset(xT_sb[:, :, 512:516], 0.0)
                for qc in range(1, QC):
                    for t in range(KT):
                        xt_dma(nc.sync if t % 2 else nc.gpsimd, t, qc)

            # ---------------- filler units ----------------
            def emit_q(qc2, mt, w_sb=wq_sb, dst=None):
                dst = qT_sb if dst is None else dst
                ps = ps_m_pool.tile([128, 512], F32, tag="misc", name="mps")
                for kt in range(KT):
                    nc.tensor.matmul(
                        ps,
                        lhsT=w_sb[:, kt, 128 * mt : 128 * (mt + 1)],
                        rhs=xT_sb[:, kt, 512 * qc2 : 512 * (qc2 + 1)],
                        start=(kt == 0),
                        stop=(kt == KT - 1),
                    )
                nc.vector.tensor_copy(
                    dst[:, mt, 512 * qc2 : 512 * (qc2 + 1)], ps
                )

            def emit_k(kc, mt):
                emit_q(kc, mt, w_sb=wk_sb, dst=kT_sb)

            def emit_v(st):
                ps = ps_m_pool.tile([128, 512], F32, tag="misc", name="mps")
                psv = ps[:, 0:DH]
                for kt in range(KT):
                    nc.tensor.matmul(
                        psv,
                        lhsT=xT_sb[:, kt, 128 * st : 128 * (st + 1)],
                        rhs=wv_sb[:, kt, :],
                        start=(kt == 0),
                        stop=(kt == KT - 1),
                    )
                # even heads 0,2 -> v_sb offsets 0,193; odd heads 1,3 -> 129,322
                ev = bass.AP(
                    tensor=v_sb.tensor,
                    offset=v_sb[:, st, 0:1].offset,
                    ap=[v_sb.ap[0], [193, 2], [1, HD]],
                )
                od = bass.AP(
                    tensor=v_sb.tensor,
                    offset=v_sb[:, st, 129:130].offset,
                    ap=[v_sb.ap[0], [193, 2], [1, HD]],
                )
                in_ev = bass.AP(
                    tensor=ps.tensor,
                    offset=ps[:, 0:1].offset,
                    ap=[ps.ap[0], [2 * HD, 2], [1, HD]],
                )
                in_od = bass.AP(
                    tensor=ps.tensor,
                    offset=ps[:, HD : HD + 1].offset,
                    ap=[ps.ap[0], [2 * HD, 2], [1, HD]],
                )
                nc.vector.tensor_copy(ev, in_ev)
                nc.vector.tensor_copy(od, in_od)

            def emit_y(st, half):
                ps = ps_m_pool.tile([128, 512], F32, tag="misc", name="mps")
                for kt2 in range(2):
                    nc.tensor.matmul(
                        ps,
                        lhsT=ctxT_sb[:, kt2, 128 * st : 128 * (st + 1)],
                        rhs=wo_sb[:, kt2, 512 * half : 512 * (half + 1)],
                        start=(kt2 == 0),
                        stop=(kt2 == 1),
                    )
                ysb = io_pool.tile([128, 512], BF16)
                nc.vector.tensor_copy(ysb, ps)
                nc.sync.dma_start(
                    out=y_r[st][:, 512 * half : 512 * (half + 1)], in_=ysb
                )

            def emit_sx(mt, kt):
                # chunk-3 S pair + exp for a non-diagonal k-tile (kt < 8),
                # stored in ex3_sb for consumption during chunk 3
                ps_s = ps_s_pool.tile([128, 1024], F32, tag="s", name="sx")
                for parity in (0, 1):
                    pofs = 64 * parity
                    nc.tensor.matmul(
                        ps_s[:, 512 * parity : 512 * (parity + 1)],
                        lhsT=kT_sb[
                            pofs : pofs + 64, mt, 128 * kt : 128 * (kt + 1)
                        ],
                        rhs=qT_sb[pofs : pofs + 64, mt, 512 * 3 : N],
                        start=True,
                        stop=True,
                    )
                nc.scalar.activation(
                    ex3_sb[:, 8 * mt + kt, :],
                    ps_s,
                    mybir.ActivationFunctionType.Exp,
                    scale=1.0 / np.sqrt(HD),
                )

            def emit(unit):
                if unit is None:
                    return
                kind = unit[0]
                if kind == "q":
                    emit_q(unit[1], unit[2])
                elif kind == "k":
                    emit_k(unit[1], unit[2])
                elif kind == "v":
                    emit_v(unit[1])
                elif kind == "y":
                    emit_y(unit[1], unit[2])
                elif kind == "sx":
                    emit_sx(unit[1], unit[2])

            # ---------------- normalize chain ----------------
            # Runs at the start of the *following* stream, spread over its
            # first three iterations so no engine queue head-of-line blocks:
            # the U banks are freed once step3's multiplies read them, just
            # before the next stream's first PV (LAG=2) needs them.
            def make_norm_steps(qc, mt, ue, uo):
                rtmp = small_pool.tile([128, 1024], BF16, tag="rtmp")
                rb = small_pool.tile([128, 512], F32, tag="rb")

                def s1():
                    # denominator rows -> SBUF (bf16)
                    nc.vector.tensor_copy(rtmp[64:65, 0:512], ue[64:65, :])
                    nc.vector.tensor_copy(rtmp[0:1, 512:1024], uo[0:1, :])

                def s2():
                    # partition-broadcast via two K=1 rank-1 matmuls, then 1/r
                    pb = ps_m_pool.tile([128, 512], F32, tag="misc", name="pb")
                    nc.tensor.matmul(
                        pb[0:64, :],
                        lhsT=ones_sb[64:65, 0:64],
                        rhs=rtmp[64:65, 0:512],
                        start=True,
                        stop=True,
                    )
                    nc.tensor.matmul(
                        pb[64:128, :],
                        lhsT=ones_sb[0:1, 0:64],
                        rhs=rtmp[0:1, 512:1024],
                        start=True,
                        stop=True,
                    )
                    nc.vector.reciprocal_approx_fast(out=rb, in_=pb)

                def s3():
                    nc.vector.tensor_mul(
                        ctxT_sb[0:64, mt, 512 * qc : 512 * (qc + 1)],
                        ue[0:64, :],
                        rb[0:64, :],
                    )
                    nc.vector.tensor_mul(
                        ctxT_sb[64:128, mt, 512 * qc : 512 * (qc + 1)],
                        uo[64:128, :],
                        rb[64:128, :],
                    )

                return [(0, s1), (1, s2), (2, s3)]

            # ---------------- schedules ----------------
            # preamble: Q/K projections for chunk-0/mt-0, interleaved per
            # k-tile so both consume each xT quarter as its DMA lands (V
            # units wait for wv, which lands later -- they go in c0/m0 slots)
            psq = ps_m_pool.tile([128, 512], F32, tag="misc", name="mps")
            psk = ps_m_pool.tile([128, 512], F32, tag="misc", name="mps")
            for kt in range(KT):
                for w_sb, ps in ((wq_sb, psq), (wk_sb, psk)):
                    nc.tensor.matmul(
                        ps,
                        lhsT=w_sb[:, kt, 0:128],
                        rhs=xT_sb[:, kt, 0:512],
                        start=(kt == 0),
                        stop=(kt == KT - 1),
                    )
            nc.vector.tensor_copy(qT_sb[:, 0, 0:512], psq)
            nc.vector.tensor_copy(kT_sb[:, 0, 0:512], psk)
            emit_xt_rest()

            # one filler per attention iteration (none at iteration 0 of
            # streams with a pending normalize chain); Y units of chunk qc-1
            # may appear in stream (qc, m0) only from iteration 3 on (the
            # previous normalize's last step lands at iteration 2)
            fifo = deque(
                [
                    # c0 m0 (iters 0-3, 4 slots)
                    ("v", 0), ("v", 1), ("v", 2), ("v", 3),
                    # c0 m1 (iters 1-3, 3)
                    ("v", 4), ("v", 5), ("q", 1, 0),
                    # c1 m0 (7)
                    ("k", 1, 0), ("q", 1, 1), ("k", 1, 1), ("v", 6),
                    ("v", 7), ("y", 0, 0), ("y", 0, 1),
                    # c1 m1 (7)
                    ("q", 2, 0), ("k", 2, 0), ("y", 1, 0), ("y", 1, 1),
                    ("y", 2, 0), ("y", 2, 1), ("v", 8),
                    # c2 m0 (11)
                    ("q", 2, 1), ("k", 2, 1), ("q", 3, 0), ("k", 3, 0),
                    ("y", 3, 0), ("y", 3, 1), ("v", 9), ("v", 10),
                    ("v", 11), ("y", 4, 0), ("y", 4, 1),
                    # c2 m1 (11)
                    ("q", 3, 1), ("k", 3, 1), ("y", 5, 0), ("y", 5, 1),
                    ("y", 6, 0), ("y", 6, 1), ("y", 7, 0), ("y", 7, 1),
                    ("v", 12), ("v", 13), ("v", 14),
                    # c3 m0 (15)
                    ("v", 15), None, None,
                    ("y", 8, 0), ("y", 8, 1), ("y", 9, 0), ("y", 9, 1),
                    None, None, None, None, None, None, None, None,
                    # c3 m1 (15): Y units late, where the stream runs dry
                ]
                + [None] * 11
                + [("y", 10, 0), ("y", 10, 1), None, None]
                # held back for the tail (interleave with the last normalize)
                + [("y", 11, 0), ("y", 11, 1)]
            )

            # extra chunk-3 exp-precompute units on specific (stream, iter)
            # slots: c2/m0 iters 5-11 get head-pair 0, c2/m1 iter 1 the last
            # of pair 0 and iters 3-10 head-pair 1
            extras = {}
            extras[(0, 2)] = ("q", 0, 1)
            extras[(0, 3)] = ("k", 0, 1)
            for j in range(7):
                extras[(4, 5 + j)] = ("sx", 0, j)
            extras[(5, 1)] = ("sx", 0, 7)
            for j in range(8):
                extras[(5, 3 + j)] = ("sx", 1, j)

            # ---------------- attention streams ----------------
            pending = []  # normalize steps due in the current stream
            streams = [(qc, mt) for qc in range(QC) for mt in range(2)]
            for si, (qc, mt) in enumerate(streams):
                nkt = 4 * (qc + 1)
                pv_q = []  # (ex tile, kt, off)

                ue = ps_u_pool.tile([128, 512], F32, tag="u", name="ue")
                uo = ps_u_pool.tile([128, 512], F32, tag="u", name="uo")
                uu = {0: ue, 1: uo}

                def emit_pv(ex_prev, kt_prev, off_prev):
                    for parity in (0, 1):
                        head = 2 * mt + parity
                        blo, bhi = V_BLK[head]
                        ex_ap = bass.AP(
                            tensor=ex_prev.tensor,
                            offset=ex_prev[
                                :,
                                512 * parity + off_prev : 512 * parity
                                + off_prev
                                + 1,
                            ].offset,
                            ap=[ex_prev.ap[0], [1, 512 - off_prev]],
                        )
                        nc.tensor.matmul(
                            uu[parity][0 : bhi - blo, off_prev:512],
                            lhsT=v_sb[:, kt_prev, blo:bhi],
                            rhs=ex_ap,
                            start=(kt_prev == 0),
                            stop=(kt_prev == nkt - 1),
                            skip_group_check=True,
                        )

                def emit_s_exp_mask(kt):
                    di = kt - 4 * qc
                    off = 128 * di if di >= 0 else 0
                    # S^T for both heads of the pair; the two K=64 matmuls
                    # occupy PE row strips h0/h64 and run concurrently
                    ps_s = ps_s_pool.tile([128, 1024], F32, tag="s", name="s")
                    for parity in (0, 1):
                        pofs = 64 * parity
                        nc.tensor.matmul(
                            ps_s[:, 512 * parity + off : 512 * (parity + 1)],
                            lhsT=kT_sb[
                                pofs : pofs + 64, mt, 128 * kt : 128 * (kt + 1)
                            ],
                            rhs=qT_sb[
                                pofs : pofs + 64,
                                mt,
                                512 * qc + off : 512 * (qc + 1),
                            ],
                            start=True,
                            stop=True,
                        )

                    # previous stream's normalize steps slot in right after
                    # the S pair (their PE rank-1 matmuls are tiny)
                    while pending and it >= pending[0][0]:
                        pending.pop(0)[1]()

                    ex = exp_pool.tile([128, 1024], BF16)
                    w = 512 - off
                    src_ap = bass.AP(
                        tensor=ps_s.tensor,
                        offset=ps_s[:, off : off + 1].offset,
                        ap=[ps_s.ap[0], [512, 2], [1, w]],
                    )
                    dst_ap = bass.AP(
                        tensor=ex.tensor,
                        offset=ex[:, off : off + 1].offset,
                        ap=[ex.ap[0], [512, 2], [1, w]],
                    )
                    nc.scalar.activation(
                        dst_ap,
                        src_ap,
                        mybir.ActivationFunctionType.Exp,
                        scale=1.0 / np.sqrt(HD),
                    )
                    if di >= 0:
                        # mask only the [128,128] boundary subtile (both
                        # parities in one strided multiply)
                        exm = bass.AP(
                            tensor=ex.tensor,
                            offset=ex[:, off : off + 1].offset,
                            ap=[ex.ap[0], [512, 2], [1, 128]],
                        )
                        mk = bass.AP(
                            tensor=mask_sb.tensor,
                            offset=mask_sb[:, 0:1].offset,
                            ap=[mask_sb.ap[0], [0, 2], [1, 128]],
                        )
                        nc.vector.tensor_mul(exm, exm, mk)
                    pv_q.append((ex, kt, off))

                if qc < 3:
                    for it in range(nkt):
                        # filler unit first (its DVE drain lands ahead of
                        # this iteration's mask-mul in the DVE queue); no
                        # filler at iteration 0 when a normalize is pending
                        if fifo and (it > 0 or si == 0):
                            emit(fifo.popleft())
                        if (si, it) in extras:
                            emit(extras[(si, it)])
                        emit_s_exp_mask(it)
                        if it >= LAG:
                            emit_pv(*pv_q[it - LAG])
                    for j in range(max(0, nkt - LAG), nkt):
                        emit_pv(*pv_q[j])
                else:
                    # chunk 3: k-tiles 0-7 were exp'd into ex3_sb during
                    # chunks 1-2; only k-tiles 8-15 run live.  PVs start at
                    # iteration 3 (after the previous normalize completes).
                    for it in range(nkt):
                        if fifo and it > 0:
                            emit(fifo.popleft())
                        if it < 8:
                            emit_s_exp_mask(8 + it)
                        if 3 <= it < 11:
                            ktp = it - 3
                            emit_pv(ex3_sb[:, 8 * mt + ktp, :], ktp, 0)
                        elif it >= 11:
                            emit_pv(*pv_q[it - 11])
                    for j in range(5, 8):
                        emit_pv(*pv_q[j])
                while pending:  # safety: flush any unplaced steps
                    pending.pop(0)[1]()
                pending = make_norm_steps(qc, mt, ue, uo)

            # ---------------- tail ----------------
            # c3/m1 normalize interleaved with the held-back Y units, then
            # the final chunk's output projection with drains split across
            # the Vector and Scalar engines and one whole-tile DMA per st
            pending[0][1]()
            emit(fifo.popleft())
            pending[1][1]()
            emit(fifo.popleft())
            pending[2][1]()
            # the U pool is free after the last normalize -- use its banks
            # alongside misc so four output tiles pipeline in PSUM
            for st in range(12, 16):
                halves = []
                for half in range(2):
                    if half == 0:
                        ps = ps_u_pool.tile([128, 512], F32, tag="u", name="yt")
                    else:
                        ps = ps_m_pool.tile(
                            [128, 512], F32, tag="misc", name="mps"
                        )
                    for kt2 in range(2):
                        nc.tensor.matmul(
                            ps,
                            lhsT=ctxT_sb[:, kt2, 128 * st : 128 * (st + 1)],
                            rhs=wo_sb[:, kt2, 512 * half : 512 * (half + 1)],
                            start=(kt2 == 0),
                            stop=(kt2 == 1),
                        )
                    halves.append(ps)
                ysb = io_pool.tile([128, 1024], BF16, tag="ytail")
                nc.vector.tensor_copy(ysb[:, 0:512], halves[0])
                nc.scalar.copy(ysb[:, 512:1024], halves[1])
                q_eng = nc.scalar if st % 2 else nc.sync
                q_eng.dma_start(out=y_r[st], in_=ysb)

    nc.finalize()
    return nc


_NC = None


def _get_nc():
    global _NC
    if _NC is None:
        _NC = _build_nc()
    return _NC


def kernel(x, Wq, Wk, Wv, Wo):
    x = np.asarray(x, dtype=np.float32)
    bf = ml_dtypes.bfloat16
    in_maps = []
    for c in range(NCORES):
        b, g = divmod(c, 4)
        sl = slice(g * DH, (g + 1) * DH)
        in_maps.append(
            {
                "xT": np.ascontiguousarray(x[b].T).astype(bf),
                "wq": np.ascontiguousarray(np.asarray(Wq)[:, sl]).astype(bf),
                "wk": np.ascontiguousarray(np.asarray(Wk)[:, sl]).astype(bf),
                "wv": np.ascontiguousarray(np.asarray(Wv)[:, sl]).astype(bf),
                "wo": np.ascontiguousarray(np.asarray(Wo)[sl, :]).astype(bf),
            }
        )
    global _last_in_maps
    _last_in_maps = in_maps
    res = run_bass_kernel_spmd(
        _get_nc(), in_maps, core_ids=list(range(NCORES)), trace=False
    )
    out = np.zeros((B, N, D), dtype=np.float32)
    for c in range(NCORES):
        out[c // 4] += res.results[c]["y"].astype(np.float32)
    return out


# revision 40
# speedup vs baseline: 1.0087x; 1.0087x over previous
"""Multi-head causal attention (B=2, N=2048, D=1024, H=16) on 8 TRN2 NeuronCores.

Sharding: data-parallel over batch (2) x tensor-parallel over head groups (4),
so each core handles one batch element and 4 heads (256 of the 1024 hidden
channels). Wq/Wk/Wv are column-sharded, Wo row-sharded; each core emits a
partial output [2048, 1024] (bf16) that the host sums over the 4 head groups.

Per-core dataflow (matmuls bf16 with fp32 PSUM accumulation), fully
software-pipelined so the PE never idles (which would drop it out of the
2.4 GHz p-state):

  Q^T/K^T/V projections and the output projection are *interleaved into the
  attention loop* as filler units between S/PV iterations -- chunk qc's
  attention runs while chunk qc+1's Q/K and upcoming V seq-tiles project and
  chunk qc-1's output tiles drain.

  S^T[k,q] per k-tile: two K=64 matmuls at PE row groups h0/h64 (they execute
  concurrently on the PE array), trimmed at the causal diagonal (a diagonal
  k-tile with offset di only computes q-columns [128*di, 512)).

  exp on the Scalar engine covers both heads' trimmed regions in one strided
  ACTIVATE; causal masking multiplies only the [128,128] boundary subtile by
  a single shared triangular mask.

  PV (U += V_aug^T expS) lags S by 2 k-tiles; V is stored per seq-tile with
  ones/zeros scaffolding so the softmax denominator accumulates as an extra
  U row (even head: partition 64, odd head: partition 0).

  Normalize per (chunk, head-pair): the two denominator rows are copied to
  SBUF (bf16), partition-broadcast with two K=1 rank-1 matmuls into a PSUM
  tile, reciprocal'd in one DVE op, then ctx^T = U * (1/r).  No DRAM bounce.

  Y = ctx^T^T Wo emitted per seq-tile half as filler during later chunks,
  cast to bf16 and DMA'd out (host sums partials in fp32).
"""

import sys

sys.path.insert(0, "/opt/trn_rl_repo")

from collections import deque

import numpy as np
import ml_dtypes

import concourse.bass as bass
import concourse.bacc as bacc
import concourse.mybir as mybir
from concourse.tile import TileContext
from concourse.bass_utils import run_bass_kernel_spmd

BF16 = mybir.dt.bfloat16
F32 = mybir.dt.float32

B, N, D, H = 2, 2048, 1024, 16
HD = 64          # head dim
HPC = 4          # heads per core
DH = HPC * HD    # 256 hidden channels per core
NCORES = 8
KT = D // 128    # 8 contraction tiles over D
ST = N // 128    # 16 seq tiles
QC = N // 512    # 4 q-chunks of 512
LAG = 2          # PV lags S by this many k-tiles

# v_sb per-seq-tile column layout: for each head pair, an "even" block
# [V(64) | ones(1)] (matmul M=65 -> U partitions 0..64, denom at 64) and an
# "odd" block [ones(1) | zeros(63) | V(64)] (M=128 -> U partitions 64..127
# hold data, denom at partition 0, zeros keep partitions 1..63 inert).
V_BLK = {0: (0, 65), 1: (65, 193), 2: (193, 258), 3: (258, 386)}
V_COLS = 386


def _build_nc() -> bass.Bass:
    nc = bacc.Bacc()
    xT = nc.declare_dram_parameter("xT", [D, N], BF16, isOutput=False)
    wq = nc.declare_dram_parameter("wq", [D, DH], BF16, isOutput=False)
    wk = nc.declare_dram_parameter("wk", [D, DH], BF16, isOutput=False)
    wv = nc.declare_dram_parameter("wv", [D, DH], BF16, isOutput=False)
    wo = nc.declare_dram_parameter("wo", [DH, D], BF16, isOutput=False)
    y = nc.declare_dram_parameter("y", [N, D], BF16, isOutput=True)

    xT_r = xT.rearrange("(t p) n -> t p n", p=128)
    wq_r = wq.rearrange("(t p) m -> t p m", p=128)
    wk_r = wk.rearrange("(t p) m -> t p m", p=128)
    wv_r = wv.rearrange("(t p) m -> t p m", p=128)
    wo_r = wo.rearrange("(t p) m -> t p m", p=128)
    y_r = y.rearrange("(t p) m -> t p m", p=128)

    with TileContext(nc) as tc:
        with (
            tc.tile_pool(name="const", bufs=1) as cpool,
            tc.tile_pool(name="io", bufs=3) as io_pool,
            tc.tile_pool(name="exps", bufs=10) as exp_pool,
            tc.tile_pool(name="small", bufs=2) as small_pool,
            tc.tile_pool(name="ps_s", bufs=2, space="PSUM") as ps_s_pool,
            tc.tile_pool(name="ps_u", bufs=2, space="PSUM") as ps_u_pool,
            tc.tile_pool(name="ps_m", bufs=2, space="PSUM") as ps_m_pool,
        ):
            xT_sb = cpool.tile([128, KT, N], BF16)
            wq_sb = cpool.tile([128, KT, DH], BF16)
            wk_sb = cpool.tile([128, KT, DH], BF16)
            wv_sb = cpool.tile([128, KT, DH], BF16)
            wo_sb = cpool.tile([128, 2, D], BF16)
            qT_sb = cpool.tile([128, 2, N], BF16)
            kT_sb = cpool.tile([128, 2, N], BF16)
            v_sb = cpool.tile([128, ST, V_COLS], BF16)
            ctxT_sb = cpool.tile([128, 2, N], BF16)
            mask_sb = cpool.tile([128, 128], BF16)
            ones_sb = cpool.tile([128, 512], BF16)
            # chunk-3 exp tiles for k-tiles 0-7 (both head pairs), precomputed
            # as fillers during chunks 1-2 where the Scalar engine idles
            ex3_sb = cpool.tile([128, 16, 1024], BF16)

            # Shared [128,128] causal boundary mask: keep (1.0) where q >= k.
            # Emitted before the xT DMAs so the gpsimd queue produces it early.
            nc.vector.memset(mask_sb, 1.0)
            nc.gpsimd.affine_select(
                out=mask_sb,
                in_=mask_sb,
                compare_op=mybir.AluOpType.is_ge,
                fill=0.0,
                base=0,
                pattern=[[1, 128]],
                channel_multiplier=-1,
            )
            nc.vector.memset(ones_sb, 1.0)
            # ones / zeros scaffolding of the V blocks (all seq tiles at once)
            nc.vector.memset(v_sb[:, :, 66:129], 0.0)
            nc.vector.memset(v_sb[:, :, 259:322], 0.0)
            for col in (64, 65, 257, 258):
                nc.vector.memset(v_sb[:, :, col : col + 1], 1.0)

            # Weights stream on the sync DMA queue; xT streams (chunk-0
            # quarters first, then the rest) on the gpsimd queue.  The two
            # queues run concurrently.  Few, large DMAs: each call has a
            # ~0.6us descriptor floor.
            def w_dram_ap(w_r, t0, nt):
                base = w_r[t0]
                return bass.AP(
                    tensor=base.tensor,
                    offset=base.offset,
                    ap=[base.ap[0], [128 * DH, nt], [1, DH]],
                )

            def w_sb_ap(w_sb_t, t0, nt):
                return bass.AP(
                    tensor=w_sb_t.tensor,
                    offset=w_sb_t[:, t0, 0:1].offset,
                    ap=[w_sb_t.ap[0], [DH, nt], [1, DH]],
                )

            # spread the input stream over three DMA-capable queues (each
            # queue sustains only ~195 GB/s and has a ~0.6us per-call floor):
            # sync carries wq + odd xT tiles, scalar wk, gpsimd even xT
            # tiles + wv + wo
            def xt_dma(q_eng, t, qc):
                q_eng.dma_start(
                    out=xT_sb[:, t, 512 * qc : 512 * (qc + 1)],
                    in_=xT_r[t][:, 512 * qc : 512 * (qc + 1)],
                )

            def w_half_dram(w_r, mt):
                base = w_r[0]
                return bass.AP(
                    tensor=base.tensor,
                    offset=base.offset + 128 * mt,
                    ap=[base.ap[0], [128 * DH, KT], [1, 128]],
                )

            def w_half_sb(w_sb_t, mt):
                return bass.AP(
                    tensor=w_sb_t.tensor,
                    offset=w_sb_t[:, 0, 128 * mt : 128 * mt + 1].offset,
                    ap=[w_sb_t.ap[0], [DH, KT], [1, 128]],
                )

            # the head-pair-0 halves of Wq/Wk gate the first projections --
            # land them (and chunk-0 of xT) before everything else
            nc.sync.dma_start(out=w_half_sb(wq_sb, 0), in_=w_half_dram(wq_r, 0))
            nc.scalar.dma_start(
                out=w_half_sb(wk_sb, 0), in_=w_half_dram(wk_r, 0)
            )
            for t in range(1, KT, 2):
                xt_dma(nc.sync, t, 0)
            for t in range(0, KT, 2):
                xt_dma(nc.gpsimd, t, 0)
            nc.sync.dma_start(out=w_half_sb(wq_sb, 1), in_=w_half_dram(wq_r, 1))
            nc.scalar.dma_start(
                out=w_half_sb(wk_sb, 1), in_=w_half_dram(wk_r, 1)
            )
            nc.gpsimd.dma_start(
                out=w_sb_ap(wv_sb, 0, 8), in_=w_dram_ap(wv_r, 0, 8)
            )
            for t in range(2):
                nc.gpsimd.dma_start(out=wo_sb[:, t, :], in_=wo_r[t])

            def emit_xt_rest():
                # gate: the c1-c3 xT transfers start only after this memset
                # (emitted after the preamble) so they don't steal shared
                # AXI bandwidth from the chunk-0 critical-path transfers
                nc.vector.memset(xT_sb[:, :, 512:516], 0.0)
                for qc in range(1, QC):
                    for t in range(KT):
                        xt_dma(nc.sync if t % 2 else nc.gpsimd, t, qc)

            # ---------------- filler units ----------------
            def emit_q(qc2, mt, w_sb=wq_sb, dst=None):
                dst = qT_sb if dst is None else dst
                ps = ps_m_pool.tile([128, 512], F32, tag="misc", name="mps")
                for kt in range(KT):
                    nc.tensor.matmul(
                        ps,
                        lhsT=w_sb[:, kt, 128 * mt : 128 * (mt + 1)],
                        rhs=xT_sb[:, kt, 512 * qc2 : 512 * (qc2 + 1)],
                        start=(kt == 0),
                        stop=(kt == KT - 1),
                    )
                nc.vector.tensor_copy(
                    dst[:, mt, 512 * qc2 : 512 * (qc2 + 1)], ps
                )

            def emit_k(kc, mt):
                emit_q(kc, mt, w_sb=wk_sb, dst=kT_sb)

            def emit_v(st):
                ps = ps_m_pool.tile([128, 512], F32, tag="misc", name="mps")
                psv = ps[:, 0:DH]
                for kt in range(KT):
                    nc.tensor.matmul(
                        psv,
                        lhsT=xT_sb[:, kt, 128 * st : 128 * (st + 1)],
                        rhs=wv_sb[:, kt, :],
                        start=(kt == 0),
                        stop=(kt == KT - 1),
                    )
                # even heads 0,2 -> v_sb offsets 0,193; odd heads 1,3 -> 129,322
                ev = bass.AP(
                    tensor=v_sb.tensor,
                    offset=v_sb[:, st, 0:1].offset,
                    ap=[v_sb.ap[0], [193, 2], [1, HD]],
                )
                od = bass.AP(
                    tensor=v_sb.tensor,
                    offset=v_sb[:, st, 129:130].offset,
                    ap=[v_sb.ap[0], [193, 2], [1, HD]],
                )
                in_ev = bass.AP(
                    tensor=ps.tensor,
                    offset=ps[:, 0:1].offset,
                    ap=[ps.ap[0], [2 * HD, 2], [1, HD]],
                )
                in_od = bass.AP(
                    tensor=ps.tensor,
                    offset=ps[:, HD : HD + 1].offset,
                    ap=[ps.ap[0], [2 * HD, 2], [1, HD]],
                )
                nc.vector.tensor_copy(ev, in_ev)
                nc.vector.tensor_copy(od, in_od)

            def emit_y(st, half):
                ps = ps_m_pool.tile([128, 512], F32, tag="misc", name="mps")
                for kt2 in range(2):
                    nc.tensor.matmul(
                        ps,
                        lhsT=ctxT_sb[:, kt2, 128 * st : 128 * (st + 1)],
                        rhs=wo_sb[:, kt2, 512 * half : 512 * (half + 1)],
                        start=(kt2 == 0),
                        stop=(kt2 == 1),
                    )
                ysb = io_pool.tile([128, 512], BF16)
                nc.vector.tensor_copy(ysb, ps)
                nc.sync.dma_start(
                    out=y_r[st][:, 512 * half : 512 * (half + 1)], in_=ysb
                )

            def emit_sx(mt, kt):
                # chunk-3 S pair + exp for a non-diagonal k-tile (kt < 8),
                # stored in ex3_sb for consumption during chunk 3
                ps_s = ps_s_pool.tile([128, 1024], F32, tag="s", name="sx")
                for parity in (0, 1):
                    pofs = 64 * parity
                    nc.tensor.matmul(
                        ps_s[:, 512 * parity : 512 * (parity + 1)],
                        lhsT=kT_sb[
                            pofs : pofs + 64, mt, 128 * kt : 128 * (kt + 1)
                        ],
                        rhs=qT_sb[pofs : pofs + 64, mt, 512 * 3 : N],
                        start=True,
                        stop=True,
                    )
                nc.scalar.activation(
                    ex3_sb[:, 8 * mt + kt, :],
                    ps_s,
                    mybir.ActivationFunctionType.Exp,
                    scale=1.0 / np.sqrt(HD),
                )

            def emit(unit):
                if unit is None:
                    return
                kind = unit[0]
                if kind == "q":
                    emit_q(unit[1], unit[2])
                elif kind == "k":
                    emit_k(unit[1], unit[2])
                elif kind == "v":
                    emit_v(unit[1])
                elif kind == "y":
                    emit_y(unit[1], unit[2])
                elif kind == "sx":
                    emit_sx(unit[1], unit[2])

            # ---------------- normalize chain ----------------
            # Runs at the start of the *following* stream, spread over its
            # first three iterations so no engine queue head-of-line blocks:
            # the U banks are freed once step3's multiplies read them, just
            # before the next stream's first PV (LAG=2) needs them.
            def make_norm_steps(qc, mt, ue, uo):
                rtmp = small_pool.tile([128, 1024], BF16, tag="rtmp")
                rb = small_pool.tile([128, 512], F32, tag="rb")

                def s1():
                    # denominator rows -> SBUF (bf16)
                    nc.vector.tensor_copy(rtmp[64:65, 0:512], ue[64:65, :])
                    nc.vector.tensor_copy(rtmp[0:1, 512:1024], uo[0:1, :])

                def s2():
                    # partition-broadcast via two K=1 rank-1 matmuls, then 1/r
                    pb = ps_m_pool.tile([128, 512], F32, tag="misc", name="pb")
                    nc.tensor.matmul(
                        pb[0:64, :],
                        lhsT=ones_sb[64:65, 0:64],
                        rhs=rtmp[64:65, 0:512],
                        start=True,
                        stop=True,
                    )
                    nc.tensor.matmul(
                        pb[64:128, :],
                        lhsT=ones_sb[0:1, 0:64],
                        rhs=rtmp[0:1, 512:1024],
                        start=True,
                        stop=True,
                    )
                    nc.vector.reciprocal_approx_fast(out=rb, in_=pb)

                def s3():
                    nc.vector.tensor_mul(
                        ctxT_sb[0:64, mt, 512 * qc : 512 * (qc + 1)],
                        ue[0:64, :],
                        rb[0:64, :],
                    )
                    nc.vector.tensor_mul(
                        ctxT_sb[64:128, mt, 512 * qc : 512 * (qc + 1)],
                        uo[64:128, :],
                        rb[64:128, :],
                    )

                return [(0, s1), (1, s2), (2, s3)]

            # ---------------- schedules ----------------
            # preamble: Q/K projections for chunk-0/mt-0, interleaved per
            # k-tile so both consume each xT quarter as its DMA lands (V
            # units wait for wv, which lands later -- they go in c0/m0 slots)
            psq = ps_m_pool.tile([128, 512], F32, tag="misc", name="mps")
            psk = ps_m_pool.tile([128, 512], F32, tag="misc", name="mps")
            for kt in range(KT):
                for w_sb, ps in ((wq_sb, psq), (wk_sb, psk)):
                    nc.tensor.matmul(
                        ps,
                        lhsT=w_sb[:, kt, 0:128],
                        rhs=xT_sb[:, kt, 0:512],
                        start=(kt == 0),
                        stop=(kt == KT - 1),
                    )
            nc.vector.tensor_copy(qT_sb[:, 0, 0:512], psq)
            nc.vector.tensor_copy(kT_sb[:, 0, 0:512], psk)
            emit_xt_rest()

            # one filler per attention iteration (none at iteration 0 of
            # streams with a pending normalize chain); Y units of chunk qc-1
            # may appear in stream (qc, m0) only from iteration 3 on (the
            # previous normalize's last step lands at iteration 2)
            fifo = deque(
                [
                    # c0 m0 (iters 0-3, 4 slots)
                    ("v", 0), ("v", 1), ("v", 2), ("v", 3),
                    # c0 m1 (iters 1-3, 3)
                    ("v", 4), ("v", 5), ("q", 1, 0),
                    # c1 m0 (7)
                    ("k", 1, 0), ("q", 1, 1), ("k", 1, 1), ("v", 6),
                    ("v", 7), ("y", 0, 0), ("y", 0, 1),
                    # c1 m1 (7)
                    ("q", 2, 0), ("k", 2, 0), ("y", 1, 0), ("y", 1, 1),
                    ("y", 2, 0), ("y", 2, 1), ("v", 8),
                    # c2 m0 (11)
                    ("q", 2, 1), ("k", 2, 1), ("q", 3, 0), ("k", 3, 0),
                    ("y", 3, 0), ("y", 3, 1), ("v", 9), ("v", 10),
                    ("v", 11), ("y", 4, 0), ("y", 4, 1),
                    # c2 m1 (11)
                    ("q", 3, 1), ("k", 3, 1), ("y", 5, 0), ("y", 5, 1),
                    ("y", 6, 0), ("y", 6, 1), ("y", 7, 0), ("y", 7, 1),
                    ("v", 12), ("v", 13), ("v", 14),
                    # c3 m0 (15)
                    ("v", 15), None, None,
                    ("y", 8, 0), ("y", 8, 1), ("y", 9, 0), ("y", 9, 1),
                    None, None, None, None, None, None, None, None,
                    # c3 m1 (15)
                ]
                + [("y", 10, 0), ("y", 10, 1)]
                + [None] * 13
                # held back for the tail (interleave with the last normalize)
                + [("y", 11, 0), ("y", 11, 1)]
            )

            # extra chunk-3 exp-precompute units on specific (stream, iter)
            # slots: c2/m0 iters 5-11 get head-pair 0, c2/m1 iter 1 the last
            # of pair 0 and iters 3-10 head-pair 1
            extras = {}
            extras[(0, 2)] = ("q", 0, 1)
            extras[(0, 3)] = ("k", 0, 1)
            for j in range(7):
                extras[(4, 5 + j)] = ("sx", 0, j)
            extras[(5, 1)] = ("sx", 0, 7)
            for j in range(8):
                extras[(5, 3 + j)] = ("sx", 1, j)

            # ---------------- attention streams ----------------
            pending = []  # normalize steps due in the current stream
            streams = [(qc, mt) for qc in range(QC) for mt in range(2)]
            for si, (qc, mt) in enumerate(streams):
                nkt = 4 * (qc + 1)
                pv_q = []  # (ex tile, kt, off)

                ue = ps_u_pool.tile([128, 512], F32, tag="u", name="ue")
                uo = ps_u_pool.tile([128, 512], F32, tag="u", name="uo")
                uu = {0: ue, 1: uo}

                def emit_pv(ex_prev, kt_prev, off_prev):
                    for parity in (0, 1):
                        head = 2 * mt + parity
                        blo, bhi = V_BLK[head]
                        ex_ap = bass.AP(
                            tensor=ex_prev.tensor,
                            offset=ex_prev[
                                :,
                                512 * parity + off_prev : 512 * parity
                                + off_prev
                                + 1,
                            ].offset,
                            ap=[ex_prev.ap[0], [1, 512 - off_prev]],
                        )
                        nc.tensor.matmul(
                            uu[parity][0 : bhi - blo, off_prev:512],
                            lhsT=v_sb[:, kt_prev, blo:bhi],
                            rhs=ex_ap,
                            start=(kt_prev == 0),
                            stop=(kt_prev == nkt - 1),
                            skip_group_check=True,
                        )

                def emit_s_exp_mask(kt):
                    di = kt - 4 * qc
                    off = 128 * di if di >= 0 else 0
                    # S^T for both heads of the pair; the two K=64 matmuls
                    # occupy PE row strips h0/h64 and run concurrently
                    ps_s = ps_s_pool.tile([128, 1024], F32, tag="s", name="s")
                    for parity in (0, 1):
                        pofs = 64 * parity
                        nc.tensor.matmul(
                            ps_s[:, 512 * parity + off : 512 * (parity + 1)],
                            lhsT=kT_sb[
                                pofs : pofs + 64, mt, 128 * kt : 128 * (kt + 1)
                            ],
                            rhs=qT_sb[
                                pofs : pofs + 64,
                                mt,
                                512 * qc + off : 512 * (qc + 1),
                            ],
                            start=True,
                            stop=True,
                        )

                    # previous stream's normalize steps slot in right after
                    # the S pair (their PE rank-1 matmuls are tiny)
                    while pending and it >= pending[0][0]:
                        pending.pop(0)[1]()

                    ex = exp_pool.tile([128, 1024], BF16)
                    w = 512 - off
                    src_ap = bass.AP(
                        tensor=ps_s.tensor,
                        offset=ps_s[:, off : off + 1].offset,
                        ap=[ps_s.ap[0], [512, 2], [1, w]],
                    )
                    dst_ap = bass.AP(
                        tensor=ex.tensor,
                        offset=ex[:, off : off + 1].offset,
                        ap=[ex.ap[0], [512, 2], [1, w]],
                    )
                    nc.scalar.activation(
                        dst_ap,
                        src_ap,
                        mybir.ActivationFunctionType.Exp,
                        scale=1.0 / np.sqrt(HD),
                    )
                    if di >= 0:
                        # mask only the [128,128] boundary subtile (both
                        # parities in one strided multiply)
                        exm = bass.AP(
                            tensor=ex.tensor,
                            offset=ex[:, off : off + 1].offset,
                            ap=[ex.ap[0], [512, 2], [1, 128]],
                        )
                        mk = bass.AP(
                            tensor=mask_sb.tensor,
                            offset=mask_sb[:, 0:1].offset,
                            ap=[mask_sb.ap[0], [0, 2], [1, 128]],
                        )
                        nc.vector.tensor_mul(exm, exm, mk)
                    pv_q.append((ex, kt, off))

                if qc < 3:
                    for it in range(nkt):
                        # filler unit first (its DVE drain lands ahead of
                        # this iteration's mask-mul in the DVE queue); no
                        # filler at iteration 0 when a normalize is pending
                        if fifo and (it > 0 or si == 0):
                            emit(fifo.popleft())
                        if (si, it) in extras:
                            emit(extras[(si, it)])
                        emit_s_exp_mask(it)
                        if it >= LAG:
                            emit_pv(*pv_q[it - LAG])
                    for j in range(max(0, nkt - LAG), nkt):
                        emit_pv(*pv_q[j])
                else:
                    # chunk 3: k-tiles 0-7 were exp'd into ex3_sb during
                    # chunks 1-2; only k-tiles 8-15 run live.  PVs start at
                    # iteration 3 (after the previous normalize completes).
                    for it in range(nkt):
                        if fifo and it > 0:
                            emit(fifo.popleft())
                        if it < 8:
                            emit_s_exp_mask(8 + it)
                        if 3 <= it < 11:
                            ktp = it - 3
                            emit_pv(ex3_sb[:, 8 * mt + ktp, :], ktp, 0)
                        elif it >= 11:
                            emit_pv(*pv_q[it - 11])
                    for j in range(5, 8):
                        emit_pv(*pv_q[j])
                while pending:  # safety: flush any unplaced steps
                    pending.pop(0)[1]()
                pending = make_norm_steps(qc, mt, ue, uo)

            # ---------------- tail ----------------
            # c3/m1 normalize interleaved with the held-back Y units, then
            # the final chunk's output projection with drains split across
            # the Vector and Scalar engines and one whole-tile DMA per st
            pending[0][1]()
            emit(fifo.popleft())
            pending[1][1]()
            emit(fifo.popleft())
            pending[2][1]()
            for st in range(12, 16):
                halves = []
                for half in range(2):
                    ps = ps_m_pool.tile(
                        [128, 512], F32, tag="misc", name="mps"
                    )
                    for kt2 in range(2):
                        nc.tensor.matmul(
                            ps,
                            lhsT=ctxT_sb[:, kt2, 128 * st : 128 * (st + 1)],
                            rhs=wo_sb[:, kt2, 512 * half : 512 * (half + 1)],
                            start=(kt2 == 0),
                            stop=(kt2 == 1),
                        )
                    halves.append(ps)
                ysb = io_pool.tile([128, 1024], BF16, tag="ytail")
                nc.vector.tensor_copy(ysb[:, 0:512], halves[0])
                nc.scalar.copy(ysb[:, 512:1024], halves[1])
                q_eng = nc.scalar if st % 2 else nc.sync
                q_eng.dma_start(out=y_r[st], in_=ysb)

    nc.finalize()
    return nc


_NC = None


def _get_nc():
    global _NC
    if _NC is None:
        _NC = _build_nc()
    return _NC


def kernel(x, Wq, Wk, Wv, Wo):
    x = np.asarray(x, dtype=np.float32)
    bf = ml_dtypes.bfloat16
    in_maps = []
    for c in range(NCORES):
        b, g = divmod(c, 4)
        sl = slice(g * DH, (g + 1) * DH)
        in_maps.append(
            {
                "xT": np.ascontiguousarray(x[b].T).astype(bf),
                "wq": np.ascontiguousarray(np.asarray(Wq)[:, sl]).astype(bf),
                "wk": np.ascontiguousarray(np.asarray(Wk)[:, sl]).astype(bf),
                "wv": np.ascontiguousarray(np.asarray(Wv)[:, sl]).astype(bf),
                "wo": np.ascontiguousarray(np.asarray(Wo)[sl, :]).astype(bf),
            }
        )
    global _last_in_maps
    _last_in_maps = in_maps
    res = run_bass_kernel_spmd(
        _get_nc(), in_maps, core_ids=list(range(NCORES)), trace=False
    )
    out = np.zeros((B, N, D), dtype=np.float32)
    for c in range(NCORES):
        out[c // 4] += res.results[c]["y"].astype(np.float32)
    return out
